# revision 24
# baseline (speedup 1.0000x reference)
"""Sharded multi-head attention for TRN2 (8 NeuronCores).

Problem: B=4, H=16, S=2048, DK=64 attention with boolean mask [B,1,S,S]
(True entries masked out).  The 64 (batch, head) pairs are independent:
core c handles batch c//2, heads (c%2)*8 .. (c%2)*8+8.

Two-stream software pipeline.  The scalar engine (exp) is the
bottleneck (256 x ~1us activations); everything is organized to keep it
fed:
  - TWO independent streams (pairs 0-1 and 2-3) run in alternating
    slots.  Each stream owns half of PSUM: sc [128,1024] (2 banks) +
    accA/accB [65,512] (1 bank each).  A stream's qc/pair boundary
    (epilogue, new accumulators) hides under the other stream's steady
    state, so exp rarely stalls at boundaries.
  - All inputs are DMA'd up front on two queues (qkt on sync; vp+keep
    on gpsimd), priority-ordered so slot 0 can start within ~2us and
    keep[kt] always lands before its first mask-multiply.
  - Epilogue per (head, qc): sums row + acc body copied PSUM->SBUF on
    DVE (frees the PSUM bank for the next qc fast), then
    recip(row partition-0) -> bf16 cast -> gpsimd partition_broadcast
    -> normalize on DVE -> gpsimd DMA out.  Scalar does exp ONLY.
    (gpsimd's Q7 has 1.3-2us op latency: keep it off any chain that
    gates PSUM reuse or DVE progress — only DMAs + the broadcast.)

Per-slot per-stream: exp over the pair tile [128, 1024] (heads A/B
side by side, scale=1/8 folded, no max-subtraction: scores ~ N(0,1));
QK for the next iteration (two 64-contraction matmuls tile-packed in
PE row groups 0:64 / 64:128, running concurrently); mask multiply on
DVE (bf16 2x, keep_T broadcast over the head dim); one lagged PV pair
(V' = [V | ones] so row 64 accumulates softmax denominators).

All DMAs are partition-major with >=2KB contiguous runs (host
pre-swizzles inputs, ones column baked into V').
"""

import numpy as np
import ml_dtypes
from contextlib import ExitStack

import concourse.bass as bass
import concourse.tile as tile
from concourse import bacc, mybir
from concourse.bass_utils import run_bass_kernel_spmd

B, H, S, DK = 4, 16, 2048, 64
N_CORES = 8
HPC = (B * H) // N_CORES  # heads per core = 8
NPAIR = HPC // 2

P = 128            # k-tile size / partition count
NKT = S // P       # 16 k tiles
QCH = 512          # q chunk per head (pair tile = [128, 1024] = 2 PSUM banks)
NQ = S // QCH      # 4 q chunks

BF16 = mybir.dt.bfloat16
F32 = mybir.dt.float32
BF = ml_dtypes.bfloat16

PV_LAG = 1  # PVs issue one stream-slot late (never head-of-queue stalls)


def build_nc():
    nc = bacc.Bacc(None, target_bir_lowering=False)
    # qkt[pair, 0] = [Q_A^T ; Q_B^T] stacked on partitions, [pair, 1] = K
    qkt_ext = nc.declare_dram_parameter("qkt", [NPAIR, 2, P, S], BF16, isOutput=False)
    # vp[h, p, t, :] = [V[h, t*128+p, :], 1.0]
    vp_ext = nc.declare_dram_parameter("vp", [HPC, P, NKT, DK + 1], BF16, isOutput=False)
    # keep[p, t, q] = not mask[q, t*128+p]
    keep_ext = nc.declare_dram_parameter("keep", [P, NKT, S], BF16, isOutput=False)
    # out_T[h, d, q] (host un-transposes)
    out_ext = nc.declare_dram_parameter("outT", [HPC, DK, S], F32, isOutput=True)

    with tile.TileContext(nc) as tc, ExitStack() as ctx:
        singles = ctx.enter_context(tc.tile_pool(name="singles", bufs=1))
        w_pool = ctx.enter_context(tc.tile_pool(name="wp", bufs=4))
        ep_pool = ctx.enter_context(tc.tile_pool(name="ep", bufs=1))
        ps_pool = ctx.enter_context(tc.tile_pool(name="ps", bufs=1, space="PSUM"))

        # ---- persistent SBUF tiles; all inputs prefetched up front ----
        qT, kT, vpt = {}, {}, {}
        for pr in range(NPAIR):
            qT[pr] = singles.tile([P, S], BF16, name=f"qT{pr}")
            kT[pr] = singles.tile([P, S], BF16, name=f"kT{pr}")
        for h in range(HPC):
            vpt[h] = singles.tile([P, NKT, DK + 1], BF16, name=f"vph{h}")
        keep_sb = singles.tile([P, NKT, S], BF16, name="keep_sb")

        # DMA issue itself costs ~0.7us of sequencer time per descriptor,
        # so use FEW, BIG transfers: small first chunks to unblock slot 0,
        # then whole-tensor loads.  keep kt_k is needed at ~slot k; pairs
        # 1/3 from slot 64.
        CH = S // 4
        for pr in (0, 2):
            nc.sync.dma_start(out=qT[pr][:, 0:CH], in_=qkt_ext[pr, 0, :, 0:CH])
            nc.sync.dma_start(out=kT[pr][:, 0:CH], in_=qkt_ext[pr, 1, :, 0:CH])
        for pr in (0, 2):
            nc.sync.dma_start(out=kT[pr][:, CH:S], in_=qkt_ext[pr, 1, :, CH:S])
        for pr in (0, 2):
            nc.sync.dma_start(out=qT[pr][:, CH:S], in_=qkt_ext[pr, 0, :, CH:S])
        for pr in (1, 3):
            nc.sync.dma_start(out=kT[pr], in_=qkt_ext[pr, 1])
            nc.sync.dma_start(out=qT[pr], in_=qkt_ext[pr, 0])
        for h in (0, 1, 4, 5):
            nc.gpsimd.dma_start(out=vpt[h], in_=vp_ext[h])
        nc.gpsimd.dma_start(out=keep_sb[:, 0], in_=keep_ext[:, 0])
        nc.gpsimd.dma_start(out=keep_sb[:, 1], in_=keep_ext[:, 1])
        nc.gpsimd.dma_start(out=keep_sb[:, 2:4], in_=keep_ext[:, 2:4])
        nc.gpsimd.dma_start(out=keep_sb[:, 4:8], in_=keep_ext[:, 4:8])
        for h in (2, 3, 6, 7):
            nc.gpsimd.dma_start(out=vpt[h], in_=vp_ext[h])
        nc.gpsimd.dma_start(out=keep_sb[:, 8:NKT], in_=keep_ext[:, 8:NKT])

        # ---- two interleaved streams ----
        streams = []
        for si, prs in enumerate(((0, 1), (2, 3))):
            its = [
                (pr, qc, kt)
                for pr in prs
                for qc in range(NQ)
                for kt in range(NKT)
            ]
            streams.append(
                {"si": si, "iters": its, "sc": None, "accA": None,
                 "accB": None, "pend": [], "dues": []}
            )
        NSLOT = len(streams[0]["iters"])  # 128

        def issue_qk(st, i):
            pr, qc, kt = st["iters"][i]
            si = st["si"]
            q0, k0 = qc * QCH, kt * P
            sc = ps_pool.tile(
                [P, 2 * QCH], F32, tag=f"sc{si}", name=f"sc{si}_{i}", bufs=1
            )
            nc.tensor.matmul(
                sc[:, 0:QCH],
                kT[pr][0:DK, k0 : k0 + P],
                qT[pr][0:DK, q0 : q0 + QCH],
                start=True, stop=True, tile_position=(0, 0),
            )
            nc.tensor.matmul(
                sc[:, QCH : 2 * QCH],
                kT[pr][DK : 2 * DK, k0 : k0 + P],
                qT[pr][DK : 2 * DK, q0 : q0 + QCH],
                start=True, stop=True, tile_position=(64, 0),
            )
            return sc

        def issue_pv(st, ent):
            kt, _, _, w, aA, aB, vA, vB = ent
            nc.tensor.matmul(
                aA, vA[:, kt], w[:, 0:QCH],
                start=(kt == 0), stop=(kt == NKT - 1),
            )
            nc.tensor.matmul(
                aB, vB[:, kt], w[:, QCH : 2 * QCH],
                start=(kt == 0), stop=(kt == NKT - 1),
            )

        def ep_copy(st, acc, tg):
            """free the acc PSUM bank fast: two DVE copies (standard DVE
            ops handle the partition-64 sums row; custom ops do not)"""
            si = st["si"]
            rowF = ep_pool.tile([1, QCH], F32, tag=f"row{si}{tg}", name=f"row{si}{tg}")
            nc.vector.tensor_copy(rowF, acc[DK : DK + 1, :])
            accS = ep_pool.tile([DK, QCH], F32, tag=f"accS{si}{tg}", name=f"accS{si}{tg}")
            nc.vector.tensor_copy(accS, acc[0:DK])
            return rowF, accS

        def ep_norm(st, rowacc, h, qc, tg):
            """recip of sums row, broadcast, normalize, store (off PSUM)"""
            rowF, accS = rowacc
            si = st["si"]
            q0 = qc * QCH
            recipF = ep_pool.tile([1, QCH], F32, tag=f"rF{si}{tg}", name=f"rF{si}{tg}")
            nc.vector.reciprocal_approx_fast(recipF, rowF)
            recipS = ep_pool.tile([1, QCH], BF16, tag=f"rS{si}{tg}", name=f"rS{si}{tg}")
            nc.vector.tensor_copy(recipS, recipF)
            bcS = ep_pool.tile([DK, QCH], BF16, tag=f"bc{si}{tg}", name=f"bc{si}{tg}")
            nc.gpsimd.partition_broadcast(bcS, recipS)
            outf = ep_pool.tile([DK, QCH], F32, tag=f"of{si}{tg}", name=f"of{si}{tg}")
            nc.vector.tensor_mul(outf, accS, bcS)
            nc.gpsimd.dma_start(out=out_ext[h, :, q0 : q0 + QCH], in_=outf)

        # prologue QKs
        for st in streams:
            st["sc"] = issue_qk(st, 0)

        for s in range(NSLOT):
            for st in streams:
                si = st["si"]
                pr, qc, kt = st["iters"][s]
                if kt == 0:
                    st["accA"] = ps_pool.tile(
                        [DK + 1, QCH], F32, tag=f"acc{si}A",
                        name=f"acc{si}A_{pr}_{qc}", bufs=1,
                    )
                    st["accB"] = ps_pool.tile(
                        [DK + 1, QCH], F32, tag=f"acc{si}B",
                        name=f"acc{si}B_{pr}_{qc}", bufs=1,
                    )
                w = w_pool.tile([P, 2 * QCH], BF16, tag=f"w{si}", name=f"w{si}_{s}")
                nc.scalar.activation(
                    w, st["sc"], mybir.ActivationFunctionType.Exp, scale=0.125
                )
                if s + 1 < NSLOT:
                    st["sc"] = issue_qk(st, s + 1)
                # one masked multiply over both heads: keep slice broadcast
                # (stride-0) over the head dim
                q0 = qc * QCH
                keep_slice = keep_sb[:, kt, q0 : q0 + QCH]
                keep2 = bass.AP(
                    tensor=keep_slice.tensor,
                    offset=keep_slice.offset,
                    ap=[keep_slice.ap[0], [0, 2], keep_slice.ap[1]],
                )
                w2 = w.rearrange("p (r q) -> p r q", r=2)
                nc.vector.tensor_mul(w2, w2, keep2)
                hA, hB = 2 * pr, 2 * pr + 1
                st["pend"].append(
                    (kt, pr, qc, w, st["accA"], st["accB"], vpt[hA], vpt[hB])
                )
                if len(st["pend"]) > PV_LAG:
                    ent = st["pend"].pop(0)
                    issue_pv(st, ent)
                    if ent[0] == NKT - 1:
                        # epilogue for the finished (pr, qc): free PSUM now,
                        # normalize/store over the next slots
                        _, epr, eqc, _, eA, eB, _, _ = ent

                        def mk(stc, a, b, h0, q):
                            state = {}

                            def c0():
                                state["sA"] = ep_copy(stc, a, "A")
                                state["sB"] = ep_copy(stc, b, "B")

                            def c1():
                                ep_norm(stc, state["sA"], h0, q, "A")

                            def c2():
                                ep_norm(stc, state["sB"], h0 + 1, q, "B")

                            return c0, c1, c2

                        c0, c1, c2 = mk(st, eA, eB, 2 * epr, eqc)
                        c0()
                        st["dues"] += [(s + 1, c1), (s + 2, c2)]
                while st["dues"] and st["dues"][0][0] <= s:
                    st["dues"].pop(0)[1]()

        # flush
        for st in streams:
            while st["pend"]:
                ent = st["pend"].pop(0)
                issue_pv(st, ent)
                if ent[0] == NKT - 1:
                    _, epr, eqc, _, eA, eB, _, _ = ent
                    sA = ep_copy(st, eA, "A")
                    sB = ep_copy(st, eB, "B")
                    ep_norm(st, sA, 2 * epr, eqc, "A")
                    ep_norm(st, sB, 2 * epr + 1, eqc, "B")
            for _, fn in st["dues"]:
                fn()
            st["dues"] = []
    nc.finalize()
    return nc


_NC_CACHE = {}


def get_nc():
    if "nc" not in _NC_CACHE:
        _NC_CACHE["nc"] = build_nc()
    return _NC_CACHE["nc"]


def kernel(Q, K, V, mask, _trace=False, _tmpdir=None):
    Q = np.asarray(Q, dtype=np.float32)
    K = np.asarray(K, dtype=np.float32)
    V = np.asarray(V, dtype=np.float32)
    mask = np.asarray(mask)

    in_maps = []
    for c in range(N_CORES):
        b, h0 = c // 2, (c % 2) * HPC
        # [pair, {q,k}, 128, S]: partitions 0:64 = head A dims, 64:128 = head B
        qkt = np.empty((NPAIR, 2, P, S), BF)
        qt = Q[b, h0 : h0 + HPC].transpose(0, 2, 1).reshape(NPAIR, 2 * DK, S)
        kt = K[b, h0 : h0 + HPC].transpose(0, 2, 1).reshape(NPAIR, 2 * DK, S)
        qkt[:, 0] = qt
        qkt[:, 1] = kt
        vp = np.empty((HPC, P, NKT, DK + 1), BF)
        vp[:, :, :, 0:DK] = (
            V[b, h0 : h0 + HPC].reshape(HPC, NKT, P, DK).transpose(0, 2, 1, 3)
        )
        vp[:, :, :, DK] = 1.0
        if c % 2 == 0:
            kp = (~mask[b, 0]).T  # [k, q]
            keep = np.ascontiguousarray(
                kp.reshape(NKT, P, S).transpose(1, 0, 2)
            ).astype(BF)
        in_maps.append({"qkt": qkt, "vp": vp, "keep": keep})

    nc = get_nc()
    res = run_bass_kernel_spmd(
        nc, in_maps, core_ids=list(range(N_CORES)), trace=_trace, tmpdir=_tmpdir
    )
    out = np.empty((B, H, S, DK), np.float32)
    for c in range(N_CORES):
        b, h0 = c // 2, (c % 2) * HPC
        out[b, h0 : h0 + HPC] = np.asarray(res.results[c]["outT"]).transpose(0, 2, 1)
    if _trace:
        return out, res
    return out


# revision 25
# speedup vs baseline: 1.0703x; 1.0703x over previous
"""Sharded multi-head attention for TRN2 (8 NeuronCores).

Problem: B=4, H=16, S=2048, DK=64 attention with boolean mask [B,1,S,S]
(True entries masked out).  The 64 (batch, head) pairs are independent:
core c handles batch c//2, heads (c%2)*8 .. (c%2)*8+8.

Two-stream software pipeline.  The scalar engine (exp) is the
bottleneck (256 x ~1us activations); everything is organized to keep it
fed:
  - TWO independent streams (pairs 0-1 and 2-3) run in alternating
    slots.  Each stream owns half of PSUM: sc [128,1024] (2 banks) +
    accA/accB [65,512] (1 bank each).  A stream's qc/pair boundary
    (epilogue, new accumulators) hides under the other stream's steady
    state, so exp rarely stalls at boundaries.
  - All inputs are DMA'd up front on two queues (qkt on sync; vp+keep
    on gpsimd), priority-ordered so slot 0 can start within ~2us and
    keep[kt] always lands before its first mask-multiply.
  - Epilogue per (head, qc): sums row + acc body copied PSUM->SBUF on
    DVE (frees the PSUM bank for the next qc fast), then
    recip(row partition-0) -> bf16 cast -> gpsimd partition_broadcast
    -> normalize on DVE -> gpsimd DMA out.  Scalar does exp ONLY.
    (gpsimd's Q7 has 1.3-2us op latency: keep it off any chain that
    gates PSUM reuse or DVE progress — only DMAs + the broadcast.)

Per-slot per-stream: exp over the pair tile [128, 1024] (heads A/B
side by side, scale=1/8 folded, no max-subtraction: scores ~ N(0,1));
QK for the next iteration (two 64-contraction matmuls tile-packed in
PE row groups 0:64 / 64:128, running concurrently); mask multiply on
DVE (bf16 2x, keep_T broadcast over the head dim); one lagged PV pair
(V' = [V | ones] so row 64 accumulates softmax denominators).

All DMAs are partition-major with >=2KB contiguous runs (host
pre-swizzles inputs, ones column baked into V').
"""

import numpy as np
import ml_dtypes
from contextlib import ExitStack

import concourse.bass as bass
import concourse.tile as tile
from concourse import bacc, mybir
from concourse.bass_utils import run_bass_kernel_spmd

B, H, S, DK = 4, 16, 2048, 64
N_CORES = 8
HPC = (B * H) // N_CORES  # heads per core = 8
NPAIR = HPC // 2

P = 128            # k-tile size / partition count
NKT = S // P       # 16 k tiles
QCH = 512          # q chunk per head (pair tile = [128, 1024] = 2 PSUM banks)
NQ = S // QCH      # 4 q chunks

BF16 = mybir.dt.bfloat16
F32 = mybir.dt.float32
BF = ml_dtypes.bfloat16

PV_LAG = 1  # PVs issue one stream-slot late (never head-of-queue stalls)


def build_nc():
    nc = bacc.Bacc(None, target_bir_lowering=False)
    # qkt[pair, 0] = [Q_A^T ; Q_B^T] stacked on partitions, [pair, 1] = K
    qkt_ext = nc.declare_dram_parameter("qkt", [NPAIR, 2, P, S], BF16, isOutput=False)
    # vp[h, p, t, :] = [V[h, t*128+p, :], 1.0]
    vp_ext = nc.declare_dram_parameter("vp", [HPC, P, NKT, DK + 1], BF16, isOutput=False)
    # keep[p, t, q] = not mask[q, t*128+p]
    keep_ext = nc.declare_dram_parameter("keep", [P, NKT, S], BF16, isOutput=False)
    # out_T[h, d, q] (host un-transposes)
    out_ext = nc.declare_dram_parameter("outT", [HPC, DK, S], F32, isOutput=True)

    with tile.TileContext(nc) as tc, ExitStack() as ctx:
        singles = ctx.enter_context(tc.tile_pool(name="singles", bufs=1))
        w_pool = ctx.enter_context(tc.tile_pool(name="wp", bufs=3))
        ep_pool = ctx.enter_context(tc.tile_pool(name="ep", bufs=1))
        ps_pool = ctx.enter_context(tc.tile_pool(name="ps", bufs=1, space="PSUM"))

        # ---- persistent SBUF tiles; all inputs prefetched up front ----
        qT, kT, vpt = {}, {}, {}
        for pr in range(NPAIR):
            qT[pr] = singles.tile([P, S], BF16, name=f"qT{pr}")
            kT[pr] = singles.tile([P, S], BF16, name=f"kT{pr}")
        for h in range(HPC):
            vpt[h] = singles.tile([P, NKT, DK + 1], BF16, name=f"vph{h}")
        keep_sb = singles.tile([P, NKT, S], BF16, name="keep_sb")

        # DMA issue itself costs ~0.7us of sequencer time per descriptor,
        # so use FEW, BIG transfers: small first chunks to unblock slot 0,
        # then whole-tensor loads.  keep kt_k is needed at ~slot k; pairs
        # 1/3 from slot 64.
        CH = S // 4
        for pr in (0, 2):
            nc.sync.dma_start(out=qT[pr][:, 0:CH], in_=qkt_ext[pr, 0, :, 0:CH])
            nc.sync.dma_start(out=kT[pr][:, 0:CH], in_=qkt_ext[pr, 1, :, 0:CH])
        for pr in (0, 2):
            nc.sync.dma_start(out=kT[pr][:, CH:S], in_=qkt_ext[pr, 1, :, CH:S])
        for pr in (0, 2):
            nc.sync.dma_start(out=qT[pr][:, CH:S], in_=qkt_ext[pr, 0, :, CH:S])
        for pr in (1, 3):
            nc.sync.dma_start(out=kT[pr], in_=qkt_ext[pr, 1])
            nc.sync.dma_start(out=qT[pr], in_=qkt_ext[pr, 0])
        for h in (0, 1, 4, 5):
            nc.gpsimd.dma_start(out=vpt[h], in_=vp_ext[h])
        nc.gpsimd.dma_start(out=keep_sb[:, 0], in_=keep_ext[:, 0])
        nc.gpsimd.dma_start(out=keep_sb[:, 1], in_=keep_ext[:, 1])
        nc.gpsimd.dma_start(out=keep_sb[:, 2:4], in_=keep_ext[:, 2:4])
        nc.gpsimd.dma_start(out=keep_sb[:, 4:8], in_=keep_ext[:, 4:8])
        for h in (2, 3, 6, 7):
            nc.gpsimd.dma_start(out=vpt[h], in_=vp_ext[h])
        nc.gpsimd.dma_start(out=keep_sb[:, 8:NKT], in_=keep_ext[:, 8:NKT])

        # ---- two interleaved streams ----
        streams = []
        for si, prs in enumerate(((0, 1), (2, 3))):
            its = [
                (pr, qc, kt)
                for pr in prs
                for qc in range(NQ)
                for kt in range(NKT)
            ]
            streams.append(
                {"si": si, "iters": its, "sc": None, "accA": None,
                 "accB": None, "pend": [], "dues": []}
            )
        NSLOT = len(streams[0]["iters"])  # 128

        def issue_qk(st, i):
            pr, qc, kt = st["iters"][i]
            si = st["si"]
            q0, k0 = qc * QCH, kt * P
            sc = ps_pool.tile(
                [P, 2 * QCH], F32, tag=f"sc{si}", name=f"sc{si}_{i}", bufs=1
            )
            nc.tensor.matmul(
                sc[:, 0:QCH],
                kT[pr][0:DK, k0 : k0 + P],
                qT[pr][0:DK, q0 : q0 + QCH],
                start=True, stop=True, tile_position=(0, 0),
            )
            nc.tensor.matmul(
                sc[:, QCH : 2 * QCH],
                kT[pr][DK : 2 * DK, k0 : k0 + P],
                qT[pr][DK : 2 * DK, q0 : q0 + QCH],
                start=True, stop=True, tile_position=(64, 0),
            )
            return sc

        def issue_pv(st, ent):
            kt, _, _, w, aA, aB, vA, vB = ent
            nc.tensor.matmul(
                aA, vA[:, kt], w[:, 0:QCH],
                start=(kt == 0), stop=(kt == NKT - 1),
            )
            nc.tensor.matmul(
                aB, vB[:, kt], w[:, QCH : 2 * QCH],
                start=(kt == 0), stop=(kt == NKT - 1),
            )

        def ep_copy(st, acc, tg):
            """free the acc PSUM bank fast: two DVE copies (standard DVE
            ops handle the partition-64 sums row; custom ops do not)"""
            si = st["si"]
            rowF = ep_pool.tile([1, QCH], F32, tag=f"row{si}{tg}", name=f"row{si}{tg}")
            nc.vector.tensor_copy(rowF, acc[DK : DK + 1, :])
            accS = ep_pool.tile([DK, QCH], F32, tag=f"accS{si}{tg}", name=f"accS{si}{tg}")
            nc.vector.tensor_copy(accS, acc[0:DK])
            return rowF, accS

        def ep_norm(st, rowacc, h, qc, tg):
            """recip of sums row, broadcast, normalize, store (off PSUM)"""
            rowF, accS = rowacc
            si = st["si"]
            q0 = qc * QCH
            recipF = ep_pool.tile([1, QCH], F32, tag=f"rF{si}{tg}", name=f"rF{si}{tg}")
            nc.vector.reciprocal_approx_fast(recipF, rowF)
            recipS = ep_pool.tile([1, QCH], BF16, tag=f"rS{si}{tg}", name=f"rS{si}{tg}")
            nc.vector.tensor_copy(recipS, recipF)
            bcS = ep_pool.tile([DK, QCH], BF16, tag=f"bc{si}{tg}", name=f"bc{si}{tg}")
            nc.gpsimd.partition_broadcast(bcS, recipS)
            outf = ep_pool.tile([DK, QCH], F32, tag=f"of{si}{tg}", name=f"of{si}{tg}")
            nc.vector.tensor_mul(outf, accS, bcS)
            nc.gpsimd.dma_start(out=out_ext[h, :, q0 : q0 + QCH], in_=outf)

        # prologue QKs
        for st in streams:
            st["sc"] = issue_qk(st, 0)

        for s in range(NSLOT):
            for st in streams:
                si = st["si"]
                pr, qc, kt = st["iters"][s]
                if kt == 0:
                    st["accA"] = ps_pool.tile(
                        [DK + 1, QCH], F32, tag=f"acc{si}A",
                        name=f"acc{si}A_{pr}_{qc}", bufs=1,
                    )
                    st["accB"] = ps_pool.tile(
                        [DK + 1, QCH], F32, tag=f"acc{si}B",
                        name=f"acc{si}B_{pr}_{qc}", bufs=1,
                    )
                if s % 2 == 0:
                    # one w tile spans TWO slots (kt, kt+1); 16 slots per qc
                    # is even, so a pair never straddles a qc boundary
                    st["wbig"] = w_pool.tile(
                        [P, 4 * QCH], BF16, tag=f"w{si}", name=f"w{si}_{s}"
                    )
                w = st["wbig"][:, (s % 2) * 2 * QCH : (s % 2 + 1) * 2 * QCH]
                nc.scalar.activation(
                    w, st["sc"], mybir.ActivationFunctionType.Exp, scale=0.125
                )
                if s + 1 < NSLOT:
                    st["sc"] = issue_qk(st, s + 1)
                hA, hB = 2 * pr, 2 * pr + 1
                q0 = qc * QCH
                if s % 2 == 1:
                    # one masked multiply over both slots and both heads:
                    # keep hops kt (stride S) and broadcasts heads (stride 0)
                    keep_slice = keep_sb[:, kt - 1, q0 : q0 + QCH]
                    keep4 = bass.AP(
                        tensor=keep_slice.tensor,
                        offset=keep_slice.offset,
                        ap=[keep_slice.ap[0], [S, 2], [0, 2], keep_slice.ap[1]],
                    )
                    w4 = st["wbig"].rearrange("p (t r q) -> p t r q", t=2, r=2)
                    nc.vector.tensor_mul(w4, w4, keep4)
                    st["pend"].append(
                        (kt - 1, pr, qc, st["wbig"][:, 0 : 2 * QCH],
                         st["accA"], st["accB"], vpt[hA], vpt[hB])
                    )
                    st["pend"].append(
                        (kt, pr, qc, st["wbig"][:, 2 * QCH : 4 * QCH],
                         st["accA"], st["accB"], vpt[hA], vpt[hB])
                    )
                # strictly ONE pop per slot: pairs arrive in bursts but the
                # PE sees an even 1-PV-pair-per-slot cadence (len stays 1-2)
                if len(st["pend"]) > PV_LAG:
                    ent = st["pend"].pop(0)
                    issue_pv(st, ent)
                    if ent[0] == NKT - 1:
                        # epilogue for the finished (pr, qc): free PSUM now,
                        # normalize/store over the next slots
                        _, epr, eqc, _, eA, eB, _, _ = ent

                        def mk(stc, a, b, h0, q):
                            state = {}

                            def c0():
                                state["sA"] = ep_copy(stc, a, "A")
                                state["sB"] = ep_copy(stc, b, "B")

                            def c1():
                                ep_norm(stc, state["sA"], h0, q, "A")

                            def c2():
                                ep_norm(stc, state["sB"], h0 + 1, q, "B")

                            return c0, c1, c2

                        c0, c1, c2 = mk(st, eA, eB, 2 * epr, eqc)
                        c0()
                        st["dues"] += [(s + 1, c1), (s + 2, c2)]
                while st["dues"] and st["dues"][0][0] <= s:
                    st["dues"].pop(0)[1]()

        # flush
        for st in streams:
            while st["pend"]:
                ent = st["pend"].pop(0)
                issue_pv(st, ent)
                if ent[0] == NKT - 1:
                    _, epr, eqc, _, eA, eB, _, _ = ent
                    sA = ep_copy(st, eA, "A")
                    sB = ep_copy(st, eB, "B")
                    ep_norm(st, sA, 2 * epr, eqc, "A")
                    ep_norm(st, sB, 2 * epr + 1, eqc, "B")
            for _, fn in st["dues"]:
                fn()
            st["dues"] = []
    nc.finalize()
    return nc


_NC_CACHE = {}


def get_nc():
    if "nc" not in _NC_CACHE:
        _NC_CACHE["nc"] = build_nc()
    return _NC_CACHE["nc"]


def kernel(Q, K, V, mask, _trace=False, _tmpdir=None):
    Q = np.asarray(Q, dtype=np.float32)
    K = np.asarray(K, dtype=np.float32)
    V = np.asarray(V, dtype=np.float32)
    mask = np.asarray(mask)

    in_maps = []
    for c in range(N_CORES):
        b, h0 = c // 2, (c % 2) * HPC
        # [pair, {q,k}, 128, S]: partitions 0:64 = head A dims, 64:128 = head B
        qkt = np.empty((NPAIR, 2, P, S), BF)
        qt = Q[b, h0 : h0 + HPC].transpose(0, 2, 1).reshape(NPAIR, 2 * DK, S)
        kt = K[b, h0 : h0 + HPC].transpose(0, 2, 1).reshape(NPAIR, 2 * DK, S)
        qkt[:, 0] = qt
        qkt[:, 1] = kt
        vp = np.empty((HPC, P, NKT, DK + 1), BF)
        vp[:, :, :, 0:DK] = (
            V[b, h0 : h0 + HPC].reshape(HPC, NKT, P, DK).transpose(0, 2, 1, 3)
        )
        vp[:, :, :, DK] = 1.0
        if c % 2 == 0:
            kp = (~mask[b, 0]).T  # [k, q]
            keep = np.ascontiguousarray(
                kp.reshape(NKT, P, S).transpose(1, 0, 2)
            ).astype(BF)
        in_maps.append({"qkt": qkt, "vp": vp, "keep": keep})

    nc = get_nc()
    res = run_bass_kernel_spmd(
        nc, in_maps, core_ids=list(range(N_CORES)), trace=_trace, tmpdir=_tmpdir
    )
    out = np.empty((B, H, S, DK), np.float32)
    for c in range(N_CORES):
        b, h0 = c // 2, (c % 2) * HPC
        out[b, h0 : h0 + HPC] = np.asarray(res.results[c]["outT"]).transpose(0, 2, 1)
    if _trace:
        return out, res
    return out
